# revision 16
# baseline (speedup 1.0000x reference)
"""Trainium2 Bass kernel for DiffusionGraphConv (Chebyshev K=2 graph conv).

Strategy (8 NeuronCores, batch-sharded, no collectives):
  - reference computes x0 = [N, F*B] (col = f*B + b), x1 = S@x0,
    x2 = 2*S@x1 - x0, out[b*N+n, o] = sum_{f,m} x_m[n, f*B+b] W[f*M+m, o] + bias.
  - spmm columns are independent, so shard by batch: core c owns batches
    {2c, 2c+1} -> a [N, C=128] column slice (col = beta*64 + f).
  - per-core spmm: edges (sorted by dest row) are padded/grouped into groups
    of 128 slots whose rows span < R=8 and lie in one 512-row window.
    dma_gather pulls the 128 source rows (one per partition); a PE matmul
    with a host-built "selection matrix" (vals one-hot over row offset)
    segment-sums them into PSUM [C, 512-row window].  PSUM output is the
    TRANSPOSED x_m ([C, n]) which is exactly what the final matmul needs.
  - x1 is also written back row-major (PE transpose) as gather source for
    the second spmm.  vals are pre-doubled for spmm2 and the epilogue
    subtracts x0^T to get x2^T.
  - final: out^T[(beta,o), n] = sum_m W_m^T x_m^T per 512-col chunk, plus
    bias; host transposes/concatenates the 8 core outputs.
"""

import hashlib
import sys

import numpy as np

sys.path.insert(0, "/opt/trn_rl_repo")

# ---------------------------------------------------------------- constants
N = 20000
B = 16
F = 64
K = 2
M = K + 1
OUT = 64
E = 640000

NCORES = 8
BPC = B // NCORES          # batches per core
C = BPC * F                # 128 columns per core
R = 8                      # selmat width (max row span per group)
WIN = 512                  # psum window rows (one PSUM bank of f32)
GCH = 8                    # groups per dma_gather call (<=1024 idxs: SWDGE ring cap)
NPAD = ((N + WIN - 1) // WIN) * WIN   # 20480

PHASES = 3     # debug knob: 1 = spmm1 only, 2 = +spmm2, 3 = full

_cache = {}


# ------------------------------------------------------------- host schedule
def _build_schedule(rows, cols, vals):
    """Pad/group the (row-sorted) edge list.

    Returns slot arrays (cols_pad int32, vals_pad f32, rowrel int8) plus
    per-group r0 and per-window group ranges.  Every output row gets at
    least one slot so every psum element is written.
    """
    rows = np.asarray(rows)
    cols = np.asarray(cols)
    vals = np.asarray(vals)
    row_ptr = np.searchsorted(rows, np.arange(N + 1))

    s_cols = []
    s_vals = []
    s_rowrel = []
    g_r0 = []

    cur = 128          # slots used in current group (128 => force new)
    cur_r0 = -1
    zero_c = np.zeros(1, np.int32)
    zero_v = np.zeros(1, np.float32)

    def close_group():
        nonlocal cur
        if cur_r0 >= 0 and cur < 128:
            pad = 128 - cur
            s_cols.append(np.zeros(pad, np.int32))
            s_vals.append(np.zeros(pad, np.float32))
            s_rowrel.append(np.zeros(pad, np.int8))
            cur = 128

    for r in range(N):
        lo, hi = row_ptr[r], row_ptr[r + 1]
        if hi > lo:
            ecols = cols[lo:hi].astype(np.int32)
            evals = vals[lo:hi].astype(np.float32)
        else:
            ecols, evals = zero_c, zero_v
        m = len(ecols)
        pos = 0
        while pos < m:
            if cur == 128 or r >= cur_r0 + R or (r // WIN) != (cur_r0 // WIN):
                close_group()
                cur_r0 = r
                g_r0.append(r)
                cur = 0
            take = min(128 - cur, m - pos)
            s_cols.append(ecols[pos:pos + take])
            s_vals.append(evals[pos:pos + take])
            s_rowrel.append(np.full(take, r - cur_r0, np.int8))
            cur += take
            pos += take
    close_group()

    cols_pad = np.concatenate(s_cols)
    vals_pad = np.concatenate(s_vals)
    rowrel = np.concatenate(s_rowrel)
    g_r0 = np.asarray(g_r0, np.int32)
    S = len(cols_pad)
    G = S // 128
    assert S == G * 128 and len(g_r0) == G

    g_win = g_r0 // WIN
    g_off = g_r0 - g_win * WIN
    g_rg = np.minimum(R, WIN - g_off)          # usable selmat width
    assert (rowrel < np.repeat(g_rg, 128)).all()

    nwin = NPAD // WIN
    win_g0 = np.searchsorted(g_win, np.arange(nwin))
    win_g1 = np.searchsorted(g_win, np.arange(nwin) + 1)

    return dict(cols_pad=cols_pad, vals_pad=vals_pad, rowrel=rowrel,
                g_r0=g_r0, g_off=g_off, g_rg=g_rg,
                win_g0=win_g0, win_g1=win_g1, S=S, G=G,
                maxwg=int((win_g1 - win_g0).max()))


def _selmat(sched, vals_pad):
    """[128, G*R] f32: sel[p, g*R + rr] = vals of slot (g*128+p)."""
    G = sched["G"]
    sel = np.zeros((G, 128, R), np.float32)
    s = np.arange(sched["S"])
    sel[s // 128, s % 128, sched["rowrel"]] = vals_pad
    return np.ascontiguousarray(sel.transpose(1, 0, 2).reshape(128, G * R))


def _idx_wrap(cols_pad):
    """dma_gather index layout: [128, S/16] int16, idx i at (i%16, i//16),
    replicated across the 8 Q7 cores (partition groups of 16)."""
    a = cols_pad.astype(np.int16).reshape(-1, 16).T   # [16, S/16]
    return np.ascontiguousarray(np.tile(a, (8, 1)))


# ------------------------------------------------------------ device program
def _build_program(sched):
    import concourse.bacc as bacc
    import concourse.mybir as mybir
    from concourse.tile import TileContext

    f32 = mybir.dt.float32
    i16 = mybir.dt.int16

    G = sched["G"]
    S = sched["S"]
    g_off = sched["g_off"]
    g_rg = sched["g_rg"]
    win_g0 = sched["win_g0"]
    win_g1 = sched["win_g1"]
    MAXWG = sched["maxwg"]
    nwin = NPAD // WIN

    nc = bacc.Bacc("TRN2", target_bir_lowering=False, debug=False,
                   num_devices=NCORES)

    x0_rm = nc.declare_dram_parameter("x0_rm", [NPAD, C], f32, isOutput=False)
    x0T = nc.declare_dram_parameter("x0T", [C, NPAD], f32, isOutput=False)
    idx_d = nc.declare_dram_parameter("idx", [128, S // 16], i16, isOutput=False)
    sel1_d = nc.declare_dram_parameter("sel1", [128, G * R], f32, isOutput=False)
    sel2_d = nc.declare_dram_parameter("sel2", [128, G * R], f32, isOutput=False)
    w2_d = nc.declare_dram_parameter("w2", [64, M * OUT], f32, isOutput=False)
    bias_d = nc.declare_dram_parameter("bias", [64, 1], f32, isOutput=False)
    id_d = nc.declare_dram_parameter("id128", [128, 128], f32, isOutput=False)
    out_d = nc.declare_dram_parameter("out", [128, N], f32, isOutput=True)

    x1_rm = nc.dram_tensor("x1_rm", [NPAD, C], f32)
    x1T_d = nc.dram_tensor("x1T", [C, NPAD], f32)
    x2T_d = nc.dram_tensor("x2T", [C, NPAD], f32)

    with TileContext(nc) as tc:
        with tc.tile_pool(name="const", bufs=1) as cpool, \
             tc.tile_pool(name="io", bufs=2) as iop, \
             tc.tile_pool(name="xg", bufs=2) as xgp, \
             tc.tile_pool(name="ep", bufs=3) as epp, \
             tc.tile_pool(name="fm", bufs=2) as fmp, \
             tc.tile_pool(name="px", bufs=2, space="PSUM") as pxp, \
             tc.tile_pool(name="ptr", bufs=2, space="PSUM") as ptrp, \
             tc.tile_pool(name="po", bufs=2, space="PSUM") as pop:

            ident = cpool.tile([128, 128], f32, tag="ident")
            nc.sync.dma_start(out=ident[:], in_=id_d[:])
            w2_sb = cpool.tile([64, M * OUT], f32, tag="w2")
            nc.sync.dma_start(out=w2_sb[:], in_=w2_d[:])
            bias_sb = cpool.tile([64, 1], f32, tag="bias")
            nc.sync.dma_start(out=bias_sb[:], in_=bias_d[:])

            # ---------------- spmm pass (shared emitter) ----------------
            def spmm(src_rm, sel_d, second):
                for w in range(nwin):
                    gw0, gw1 = int(win_g0[w]), int(win_g1[w])
                    if gw0 == gw1:
                        continue
                    psum = pxp.tile([128, WIN], f32, tag="px")
                    ngw = gw1 - gw0
                    # whole window's idxs + selmat staged once
                    idx_t = iop.tile([128, MAXWG * 8], i16, tag="idx")
                    nc.sync.dma_start(
                        out=idx_t[:, :ngw * 8],
                        in_=idx_d[:, gw0 * 8:gw1 * 8])
                    sel_t = iop.tile([128, MAXWG * R], f32, tag="sel")
                    nc.sync.dma_start(
                        out=sel_t[:, :ngw * R],
                        in_=sel_d[:, gw0 * R:gw1 * R])
                    for c0 in range(gw0, gw1, GCH):
                        c1 = min(c0 + GCH, gw1)
                        ng = c1 - c0
                        nidx = ng * 128
                        o8 = (c0 - gw0) * 8
                        xg = xgp.tile([128, GCH, C], f32, tag="xg")
                        nc.gpsimd.dma_gather(
                            xg[:, :ng, :], src_rm[:],
                            idx_t[:, o8:o8 + nidx // 16],
                            nidx, nidx, C, single_packet=False)
                        for j in range(ng):
                            g = c0 + j
                            off = int(g_off[g])
                            rg = int(g_rg[g])
                            jr = (g - gw0) * R
                            nc.tensor.matmul(
                                psum[:, off:off + rg],
                                xg[:, j, :],
                                sel_t[:, jr:jr + rg],
                                start=(g == gw0), stop=(g == gw1 - 1))
                    # epilogue: psum = x_m^T window [C, WIN]
                    xt_sb = epp.tile([128, WIN], f32, tag="xt")
                    if second:
                        # x2^T = 2*S*x1^T - x0^T   (vals pre-doubled)
                        x0t_t = epp.tile([128, WIN], f32, tag="x0t")
                        nc.sync.dma_start(
                            out=x0t_t[:], in_=x0T[:, w * WIN:(w + 1) * WIN])
                        nc.vector.tensor_sub(xt_sb[:], psum[:], x0t_t[:])
                        nc.sync.dma_start(
                            out=x2T_d[:, w * WIN:(w + 1) * WIN], in_=xt_sb[:])
                    else:
                        nc.vector.tensor_copy(xt_sb[:], psum[:])
                        nc.sync.dma_start(
                            out=x1T_d[:, w * WIN:(w + 1) * WIN], in_=xt_sb[:])
                        # row-major x1 for the second gather (PE transpose)
                        for j in range(WIN // 128):
                            ptr = ptrp.tile([128, 128], f32, tag="ptr")
                            nc.tensor.transpose(
                                ptr[:], xt_sb[:, j * 128:(j + 1) * 128],
                                ident[:])
                            rm_sb = epp.tile([128, 128], f32, tag="rm")
                            nc.vector.tensor_copy(rm_sb[:], ptr[:])
                            nc.sync.dma_start(
                                out=x1_rm[w * WIN + j * 128:
                                          w * WIN + (j + 1) * 128, :],
                                in_=rm_sb[:])

            spmm(x0_rm, sel1_d, second=False)
            if PHASES >= 2:
                tc.strict_bb_all_engine_barrier()
                spmm(x1_rm, sel2_d, second=True)
            if PHASES >= 3:
                tc.strict_bb_all_engine_barrier()
                # ---------------- final matmul ----------------
                xTs = (x0T, x1T_d, x2T_d)
                for cs in range(0, N, WIN):
                    nlen = min(WIN, N - cs)
                    for beta in range(BPC):
                        xts = []
                        for m in range(M):
                            xt = fmp.tile([64, WIN], f32, tag=f"fx{m}b{beta}")
                            nc.sync.dma_start(
                                out=xt[:, :nlen],
                                in_=xTs[m][beta * 64:(beta + 1) * 64,
                                           cs:cs + nlen])
                            xts.append(xt)
                        pso = pop.tile([64, WIN], f32, tag=f"po{beta}")
                        for m in range(M):
                            nc.tensor.matmul(
                                pso[:, :nlen],
                                w2_sb[:, m * OUT:(m + 1) * OUT],
                                xts[m][:, :nlen],
                                start=(m == 0),
                                stop=(m == M - 1))
                        o_sb = fmp.tile([64, WIN], f32, tag=f"os{beta}")
                        nc.vector.tensor_scalar_add(o_sb[:, :nlen],
                                                    pso[:, :nlen],
                                                    bias_sb[:, 0:1])
                        nc.sync.dma_start(
                            out=out_d[beta * 64:(beta + 1) * 64,
                                      cs:cs + nlen],
                            in_=o_sb[:, :nlen])
            else:
                # debug: dump x1T window 0 into out_d
                dbg = fmp.tile([128, WIN], f32, tag="dbg")
                nc.sync.dma_start(out=dbg[:], in_=x1T_d[:, 0:WIN])
                nc.sync.dma_start(out=out_d[:, 0:WIN], in_=dbg[:])

    nc.compile()
    return nc


# ------------------------------------------------------------------- kernel
def kernel(inputs, sp_rows, sp_cols, sp_vals, weight, biases):
    from concourse.bass_utils import run_bass_kernel_spmd

    inputs = np.asarray(inputs, np.float32)
    sp_rows = np.asarray(sp_rows, np.int32)
    sp_cols = np.asarray(sp_cols, np.int32)
    sp_vals = np.asarray(sp_vals, np.float32)
    weight = np.asarray(weight, np.float32)
    biases = np.asarray(biases, np.float32)

    key = hashlib.sha256(sp_rows.tobytes() + sp_cols.tobytes()).hexdigest()
    if key not in _cache:
        sched = _build_schedule(sp_rows, sp_cols, sp_vals)
        nc = _build_program(sched)
        _cache[key] = (sched, nc)
    sched, nc = _cache[key]

    sel1 = _selmat(sched, sched["vals_pad"])
    sel2 = _selmat(sched, 2.0 * sched["vals_pad"])
    idx = _idx_wrap(sched["cols_pad"])

    w3 = weight.reshape(F, M, OUT)
    w2 = np.ascontiguousarray(
        np.concatenate([w3[:, m, :] for m in range(M)], axis=1))  # [64, 192]
    bias2 = np.ascontiguousarray(biases[:, None])                 # [64, 1]
    id128 = np.eye(128, dtype=np.float32)

    # x0 column slices: [N, B, F] view; core c takes batches 2c, 2c+1
    xnbf = np.transpose(inputs, (1, 0, 2))   # [N, B, F]
    in_maps = []
    for c in range(NCORES):
        x0c = np.zeros((NPAD, C), np.float32)
        x0c[:N] = xnbf[:, c * BPC:(c + 1) * BPC, :].reshape(N, C)
        x0T = np.ascontiguousarray(x0c.T)
        in_maps.append(dict(x0_rm=x0c, x0T=x0T, idx=idx, sel1=sel1,
                            sel2=sel2, w2=w2, bias=bias2, id128=id128))

    global _last_in_maps
    _last_in_maps = in_maps
    res = run_bass_kernel_spmd(nc, in_maps, list(range(NCORES)))

    out = np.empty((B, N, OUT), np.float32)
    for c in range(NCORES):
        oc = res.results[c]["out"].reshape(BPC, OUT, N)
        out[c * BPC:(c + 1) * BPC] = oc.transpose(0, 2, 1)
    return out.reshape(B * N, OUT)


# revision 17
# speedup vs baseline: 2.1161x; 2.1161x over previous
"""Trainium2 Bass kernel for DiffusionGraphConv (Chebyshev K=2 graph conv).

Strategy (8 NeuronCores, batch-sharded, no collectives):
  - reference computes x0 = [N, F*B] (col = f*B + b), x1 = S@x0,
    x2 = 2*S@x1 - x0, out[b*N+n, o] = sum_{f,m} x_m[n, f*B+b] W[f*M+m, o] + bias.
  - spmm columns are independent, so shard by batch: core c owns batches
    {2c, 2c+1} -> a [N, C=128] column slice (col = beta*64 + f).
  - per-core spmm: edges (sorted by dest row) are padded/grouped into groups
    of 128 slots whose rows span < R=8 and lie in one 512-row window.
    dma_gather pulls the 128 source rows (one per partition); a PE matmul
    with a host-built "selection matrix" (vals one-hot over row offset)
    segment-sums them into PSUM [C, 512-row window].  PSUM output is the
    TRANSPOSED x_m ([C, n]) which is exactly what the final matmul needs.
  - x1 is also written back row-major (PE transpose) as gather source for
    the second spmm.  vals are pre-doubled for spmm2 and the epilogue
    subtracts x0^T to get x2^T.
  - final: out^T[(beta,o), n] = sum_m W_m^T x_m^T per 512-col chunk, plus
    bias; host transposes/concatenates the 8 core outputs.
"""

import hashlib
import sys

import numpy as np

sys.path.insert(0, "/opt/trn_rl_repo")

# ---------------------------------------------------------------- constants
N = 20000
B = 16
F = 64
K = 2
M = K + 1
OUT = 64
E = 640000

NCORES = 8
BPC = B // NCORES          # batches per core
C = BPC * F                # 128 columns per core
R = 8                      # selmat width (max row span per group)
WIN = 512                  # psum window rows (one PSUM bank of f32)
GCH = 8                    # groups per dma_gather call (<=1024 idxs: SWDGE ring cap)
NPAD = ((N + WIN - 1) // WIN) * WIN   # 20480

PHASES = 3     # debug knob: 1 = spmm1 only, 2 = +spmm2, 3 = full

_cache = {}


# ------------------------------------------------------------- host schedule
def _build_schedule(rows, cols, vals):
    """Pad/group the (row-sorted) edge list.

    Returns slot arrays (cols_pad int32, vals_pad f32, rowrel int8) plus
    per-group r0 and per-window group ranges.  Every output row gets at
    least one slot so every psum element is written.
    """
    rows = np.asarray(rows)
    cols = np.asarray(cols)
    vals = np.asarray(vals)
    row_ptr = np.searchsorted(rows, np.arange(N + 1))

    s_cols = []
    s_vals = []
    s_rowrel = []
    g_r0 = []

    cur = 128          # slots used in current group (128 => force new)
    cur_r0 = -1
    zero_c = np.zeros(1, np.int32)
    zero_v = np.zeros(1, np.float32)

    def close_group():
        nonlocal cur
        if cur_r0 >= 0 and cur < 128:
            pad = 128 - cur
            s_cols.append(np.zeros(pad, np.int32))
            s_vals.append(np.zeros(pad, np.float32))
            s_rowrel.append(np.zeros(pad, np.int8))
            cur = 128

    for r in range(N):
        lo, hi = row_ptr[r], row_ptr[r + 1]
        if hi > lo:
            ecols = cols[lo:hi].astype(np.int32)
            evals = vals[lo:hi].astype(np.float32)
        else:
            ecols, evals = zero_c, zero_v
        m = len(ecols)
        pos = 0
        while pos < m:
            if cur == 128 or r >= cur_r0 + R or (r // WIN) != (cur_r0 // WIN):
                close_group()
                cur_r0 = r
                g_r0.append(r)
                cur = 0
            take = min(128 - cur, m - pos)
            s_cols.append(ecols[pos:pos + take])
            s_vals.append(evals[pos:pos + take])
            s_rowrel.append(np.full(take, r - cur_r0, np.int8))
            cur += take
            pos += take
    close_group()

    cols_pad = np.concatenate(s_cols)
    vals_pad = np.concatenate(s_vals)
    rowrel = np.concatenate(s_rowrel)
    g_r0 = np.asarray(g_r0, np.int32)
    S = len(cols_pad)
    G = S // 128
    assert S == G * 128 and len(g_r0) == G

    g_win = g_r0 // WIN
    g_off = g_r0 - g_win * WIN
    g_rg = np.minimum(R, WIN - g_off)          # usable selmat width
    assert (rowrel < np.repeat(g_rg, 128)).all()

    nwin = NPAD // WIN
    win_g0 = np.searchsorted(g_win, np.arange(nwin))
    win_g1 = np.searchsorted(g_win, np.arange(nwin) + 1)

    return dict(cols_pad=cols_pad, vals_pad=vals_pad, rowrel=rowrel,
                g_r0=g_r0, g_off=g_off, g_rg=g_rg,
                win_g0=win_g0, win_g1=win_g1, S=S, G=G,
                maxwg=int((win_g1 - win_g0).max()))


def _selmat(sched, vals_pad):
    """[128, G*R] f32: sel[p, g*R + rr] = vals of slot (g*128+p)."""
    G = sched["G"]
    sel = np.zeros((G, 128, R), np.float32)
    s = np.arange(sched["S"])
    sel[s // 128, s % 128, sched["rowrel"]] = vals_pad
    return np.ascontiguousarray(sel.transpose(1, 0, 2).reshape(128, G * R))


def _idx_wrap(cols_pad):
    """dma_gather index layout: [128, S/16] int16, idx i at (i%16, i//16),
    replicated across the 8 Q7 cores (partition groups of 16)."""
    a = cols_pad.astype(np.int16).reshape(-1, 16).T   # [16, S/16]
    return np.ascontiguousarray(np.tile(a, (8, 1)))


# ------------------------------------------------------------ device program
def _build_program(sched):
    import concourse.bacc as bacc
    import concourse.mybir as mybir
    from concourse.tile import TileContext

    f32 = mybir.dt.float32
    i16 = mybir.dt.int16

    G = sched["G"]
    S = sched["S"]
    g_off = sched["g_off"]
    g_rg = sched["g_rg"]
    win_g0 = sched["win_g0"]
    win_g1 = sched["win_g1"]
    MAXWG = sched["maxwg"]
    nwin = NPAD // WIN

    nc = bacc.Bacc("TRN2", target_bir_lowering=False, debug=False,
                   num_devices=NCORES, num_swdge_queues=2)

    x0_rm = nc.declare_dram_parameter("x0_rm", [NPAD, C], f32, isOutput=False)
    x0T = nc.declare_dram_parameter("x0T", [C, NPAD], f32, isOutput=False)
    idx_d = nc.declare_dram_parameter("idx", [128, S // 16], i16, isOutput=False)
    sel1_d = nc.declare_dram_parameter("sel1", [128, G * R], f32, isOutput=False)
    sel2_d = nc.declare_dram_parameter("sel2", [128, G * R], f32, isOutput=False)
    w2_d = nc.declare_dram_parameter("w2", [64, M * OUT], f32, isOutput=False)
    bias_d = nc.declare_dram_parameter("bias", [64, 1], f32, isOutput=False)
    id_d = nc.declare_dram_parameter("id128", [128, 128], f32, isOutput=False)
    out_d = nc.declare_dram_parameter("out", [128, N], f32, isOutput=True)

    x1_rm = nc.dram_tensor("x1_rm", [NPAD, C], f32)
    x1T_d = nc.dram_tensor("x1T", [C, NPAD], f32)
    x2T_d = nc.dram_tensor("x2T", [C, NPAD], f32)

    with TileContext(nc) as tc:
        with tc.tile_pool(name="const", bufs=1) as cpool, \
             tc.tile_pool(name="io", bufs=2) as iop, \
             tc.tile_pool(name="xg", bufs=4) as xgp, \
             tc.tile_pool(name="ep", bufs=3) as epp, \
             tc.tile_pool(name="fm", bufs=2) as fmp, \
             tc.tile_pool(name="px", bufs=2, space="PSUM") as pxp, \
             tc.tile_pool(name="ptr", bufs=2, space="PSUM") as ptrp, \
             tc.tile_pool(name="po", bufs=2, space="PSUM") as pop:

            ident = cpool.tile([128, 128], f32, tag="ident")
            nc.sync.dma_start(out=ident[:], in_=id_d[:])
            w2_sb = cpool.tile([64, M * OUT], f32, tag="w2")
            nc.sync.dma_start(out=w2_sb[:], in_=w2_d[:])
            bias_sb = cpool.tile([64, 1], f32, tag="bias")
            nc.sync.dma_start(out=bias_sb[:], in_=bias_d[:])

            # ---------------- spmm pass (shared emitter) ----------------
            def spmm(src_rm, sel_d, second):
                for w in range(nwin):
                    gw0, gw1 = int(win_g0[w]), int(win_g1[w])
                    if gw0 == gw1:
                        continue
                    psum = pxp.tile([128, WIN], f32, tag="px")
                    ngw = gw1 - gw0
                    # whole window's idxs + selmat staged once
                    idx_t = iop.tile([128, MAXWG * 8], i16, tag="idx")
                    nc.sync.dma_start(
                        out=idx_t[:, :ngw * 8],
                        in_=idx_d[:, gw0 * 8:gw1 * 8])
                    sel_t = iop.tile([128, MAXWG * R], f32, tag="sel")
                    nc.sync.dma_start(
                        out=sel_t[:, :ngw * R],
                        in_=sel_d[:, gw0 * R:gw1 * R])
                    for qi, c0 in enumerate(range(gw0, gw1, GCH)):
                        c1 = min(c0 + GCH, gw1)
                        ng = c1 - c0
                        nidx = ng * 128
                        o8 = (c0 - gw0) * 8
                        xg = xgp.tile([128, GCH, C], f32, tag="xg")
                        nc.gpsimd.dma_gather(
                            xg[:, :ng, :], src_rm[:],
                            idx_t[:, o8:o8 + nidx // 16],
                            nidx, nidx, C, single_packet=False,
                            queue_num=qi % 2)
                        for j in range(ng):
                            g = c0 + j
                            off = int(g_off[g])
                            rg = int(g_rg[g])
                            jr = (g - gw0) * R
                            nc.tensor.matmul(
                                psum[:, off:off + rg],
                                xg[:, j, :],
                                sel_t[:, jr:jr + rg],
                                start=(g == gw0), stop=(g == gw1 - 1))
                    # epilogue: psum = x_m^T window [C, WIN]
                    xt_sb = epp.tile([128, WIN], f32, tag="xt")
                    if second:
                        # x2^T = 2*S*x1^T - x0^T   (vals pre-doubled)
                        x0t_t = epp.tile([128, WIN], f32, tag="x0t")
                        nc.sync.dma_start(
                            out=x0t_t[:], in_=x0T[:, w * WIN:(w + 1) * WIN])
                        nc.vector.tensor_sub(xt_sb[:], psum[:], x0t_t[:])
                        nc.sync.dma_start(
                            out=x2T_d[:, w * WIN:(w + 1) * WIN], in_=xt_sb[:])
                    else:
                        nc.vector.tensor_copy(xt_sb[:], psum[:])
                        nc.sync.dma_start(
                            out=x1T_d[:, w * WIN:(w + 1) * WIN], in_=xt_sb[:])
                        # row-major x1 for the second gather (PE transpose)
                        for j in range(WIN // 128):
                            ptr = ptrp.tile([128, 128], f32, tag="ptr")
                            nc.tensor.transpose(
                                ptr[:], xt_sb[:, j * 128:(j + 1) * 128],
                                ident[:])
                            rm_sb = epp.tile([128, 128], f32, tag="rm")
                            nc.vector.tensor_copy(rm_sb[:], ptr[:])
                            nc.sync.dma_start(
                                out=x1_rm[w * WIN + j * 128:
                                          w * WIN + (j + 1) * 128, :],
                                in_=rm_sb[:])

            spmm(x0_rm, sel1_d, second=False)
            if PHASES >= 2:
                tc.strict_bb_all_engine_barrier()
                spmm(x1_rm, sel2_d, second=True)
            if PHASES >= 3:
                tc.strict_bb_all_engine_barrier()
                # ---------------- final matmul ----------------
                xTs = (x0T, x1T_d, x2T_d)
                for cs in range(0, N, WIN):
                    nlen = min(WIN, N - cs)
                    for beta in range(BPC):
                        xts = []
                        for m in range(M):
                            xt = fmp.tile([64, WIN], f32, tag=f"fx{m}b{beta}")
                            nc.sync.dma_start(
                                out=xt[:, :nlen],
                                in_=xTs[m][beta * 64:(beta + 1) * 64,
                                           cs:cs + nlen])
                            xts.append(xt)
                        pso = pop.tile([64, WIN], f32, tag=f"po{beta}")
                        for m in range(M):
                            nc.tensor.matmul(
                                pso[:, :nlen],
                                w2_sb[:, m * OUT:(m + 1) * OUT],
                                xts[m][:, :nlen],
                                start=(m == 0),
                                stop=(m == M - 1))
                        o_sb = fmp.tile([64, WIN], f32, tag=f"os{beta}")
                        nc.vector.tensor_scalar_add(o_sb[:, :nlen],
                                                    pso[:, :nlen],
                                                    bias_sb[:, 0:1])
                        nc.sync.dma_start(
                            out=out_d[beta * 64:(beta + 1) * 64,
                                      cs:cs + nlen],
                            in_=o_sb[:, :nlen])
            else:
                # debug: dump x1T window 0 into out_d
                dbg = fmp.tile([128, WIN], f32, tag="dbg")
                nc.sync.dma_start(out=dbg[:], in_=x1T_d[:, 0:WIN])
                nc.sync.dma_start(out=out_d[:, 0:WIN], in_=dbg[:])

    nc.compile()
    return nc


# ------------------------------------------------------------------- kernel
def kernel(inputs, sp_rows, sp_cols, sp_vals, weight, biases):
    from concourse.bass_utils import run_bass_kernel_spmd

    inputs = np.asarray(inputs, np.float32)
    sp_rows = np.asarray(sp_rows, np.int32)
    sp_cols = np.asarray(sp_cols, np.int32)
    sp_vals = np.asarray(sp_vals, np.float32)
    weight = np.asarray(weight, np.float32)
    biases = np.asarray(biases, np.float32)

    key = hashlib.sha256(sp_rows.tobytes() + sp_cols.tobytes()).hexdigest()
    if key not in _cache:
        sched = _build_schedule(sp_rows, sp_cols, sp_vals)
        nc = _build_program(sched)
        _cache[key] = (sched, nc)
    sched, nc = _cache[key]

    sel1 = _selmat(sched, sched["vals_pad"])
    sel2 = _selmat(sched, 2.0 * sched["vals_pad"])
    idx = _idx_wrap(sched["cols_pad"])

    w3 = weight.reshape(F, M, OUT)
    w2 = np.ascontiguousarray(
        np.concatenate([w3[:, m, :] for m in range(M)], axis=1))  # [64, 192]
    bias2 = np.ascontiguousarray(biases[:, None])                 # [64, 1]
    id128 = np.eye(128, dtype=np.float32)

    # x0 column slices: [N, B, F] view; core c takes batches 2c, 2c+1
    xnbf = np.transpose(inputs, (1, 0, 2))   # [N, B, F]
    in_maps = []
    for c in range(NCORES):
        x0c = np.zeros((NPAD, C), np.float32)
        x0c[:N] = xnbf[:, c * BPC:(c + 1) * BPC, :].reshape(N, C)
        x0T = np.ascontiguousarray(x0c.T)
        in_maps.append(dict(x0_rm=x0c, x0T=x0T, idx=idx, sel1=sel1,
                            sel2=sel2, w2=w2, bias=bias2, id128=id128))

    global _last_in_maps
    _last_in_maps = in_maps
    res = run_bass_kernel_spmd(nc, in_maps, list(range(NCORES)))

    out = np.empty((B, N, OUT), np.float32)
    for c in range(NCORES):
        oc = res.results[c]["out"].reshape(BPC, OUT, N)
        out[c * BPC:(c + 1) * BPC] = oc.transpose(0, 2, 1)
    return out.reshape(B * N, OUT)
